# revision 19
# baseline (speedup 1.0000x reference)
"""Trainium2 Bass kernel for nn_BertTransformerWSD.

Takes FULL inputs, shards batch over 8 NeuronCores (4 sequences/core),
runs a fused transformer kernel per core, gathers full output.

Numerics: bf16 matmuls with fp32 PSUM accumulate everywhere except the
precision-critical path (segment-mean pooling and layer-1 Q/K
projections + QK^T run in fp32r) -- layer-1 attention logits are
O(+-600) and softmax is near-argmax, so S needs absolute accuracy ~0.1.

Structure (per core, 4 seqs x 128 words = 512 tokens):
- pooling: token-major matmuls (A^T stationary, x moving; f32r at
  >=256-wide free dim runs at full PE rate), transpose back to
  feature-major h32 (f32r) + h16 (bf16); V-projection of layer 0 is
  interleaved into the pooling loop per seq.
- attention: layer-0 QK^T is pair-batched over 2 seqs (256-wide moving
  window) so f32r runs at 1 cyc/row; key-mask bias added via small bf16
  matmuls; softmax = DVE max (layer 0 only) -> Act exp with
  per-partition bias and accum_out denominators -> DVE reciprocal +
  normalize; PE transposes P, then AV.
- LN1 is reduced to mean-centering only: LayerNorm is invariant to a
  per-token affine and ReLU is positively homogeneous, so the rstd
  scale cancels inside LN2 (valid because b1=b2=0 and ln affine is
  identity for this problem; falls back to full LN otherwise).
- LN2: feature sums accumulated on PE while Wo/FFN2 still run; Pool
  engine broadcasts rstd/-mu*rstd; bf16 normalize on DVE.
- weight DMAs issued from the (otherwise idle) Pool engine queue,
  input/output DMAs from SP, to avoid single-queue serialization.
- vocab projection: exact 5000 columns (last chunk 392 wide).
"""
import os
import numpy as np
import ml_dtypes

# ---- model constants (hardcoded; must match reference.py) ----
B, S, T = 32, 256, 128
D_BERT, D_POS, D = 768, 256, 1024
H, DH, FF = 16, 64, 4096
NL = 2
NSENSE, NPOS = 5000, 20
SCALE = float(np.sqrt(D))
ATTN_SCALE = 1.0 / float(np.sqrt(DH))

NCORES = 8
BPC = B // NCORES           # sequences per core = 4
NTOK = BPC * T              # tokens per core = 512
KD = D // 128               # 8 k-tiles over D
MF = FF // 128              # 32 m-chunks over FF
NSP = 5120                  # padded NSENSE (10 x 512)
NCH = NSP // 512            # 10 sense chunks

BF16 = ml_dtypes.bfloat16

_BUILD_CACHE = {}


# ---------------------------------------------------------------------------
# Tile/walrus compatibility patches
# ---------------------------------------------------------------------------

def _install_patches():
    import concourse.mybir as mybir
    import concourse.tile as tile

    if getattr(tile.TileContext, "_wsd_patched", False):
        return

    def _patched_drain_and_barrier(self, tick_clock, wait_clock):
        # walrus in this container accepts at most ONE sem wait per
        # instruction; the stock exit drain carries one wait per active
        # logical processor.  Split them across SP nops.
        from concourse.tile import ScopedClock
        nc = self.nc
        probe = nc.sync.nop()
        wait_clock.add_sem_waits(probe.ins,
                                 ScopedClock({None: tick_clock.global_clock}))
        si = probe.ins.sync_info
        waits = list(si.on_wait) if si is not None and si.on_wait else []
        if len(waits) > 1:
            probe.ins.sync_info = mybir.SyncInfo(on_wait=waits[:1], on_update=[])
            for w in waits[1:]:
                n2 = nc.sync.nop()
                n2.ins.sync_info = mybir.SyncInfo(on_wait=[w], on_update=[])
        nc.sync.drain()
        nc.all_engine_barrier()
        assert self.sems is not None
        popped = nc._tile_sem_poison_stack.pop()
        assert popped is self._sem_poison
        nc.clear_and_free_semaphores(list(self.sems.allocated().values()))
        nc.all_engine_barrier()

    tile.TileContext._drain_and_barrier = _patched_drain_and_barrier
    tile.TileContext._wsd_patched = True


def _split_multi_waits(nc):
    """Safety net: split any instruction carrying >1 sem waits into
    engine-matched NoOps (sequential waits == one multi-wait)."""
    import concourse.mybir as mybir
    n = 0
    for func in nc.m.functions:
        for blk in func.blocks:
            insts = list(blk.instructions)
            rebuilt = []
            changed = False
            for inst in insts:
                si = inst.sync_info
                waits = list(si.on_wait) if si is not None and si.on_wait else []
                if len(waits) > 1:
                    for w in waits[:-1]:
                        nop = mybir.InstNoOp(name=f"I-wsplit-{n}", ins=[], outs=[])
                        n += 1
                        nop.engine = inst.engine
                        nop.sync_info = mybir.SyncInfo(on_wait=[w], on_update=[])
                        nc.register_instruction(nop, overwrite=True)
                        rebuilt.append(nop)
                    inst.sync_info = mybir.SyncInfo(
                        on_wait=[waits[-1]],
                        on_update=list(si.on_update) if si.on_update else [])
                    changed = True
                rebuilt.append(inst)
            if changed:
                while len(blk.instructions):
                    blk.instructions.pop()
                for i in rebuilt:
                    blk.instructions.append(i)


# ---------------------------------------------------------------------------
# Device kernel build
# ---------------------------------------------------------------------------

def _build(flags):
    """flags: dict of bools {has_bqk, has_bv, ..., has_gb}."""
    import concourse.bass as bass
    import concourse.mybir as mybir
    import concourse.tile as tile
    from concourse import library_config

    _install_patches()
    dt = mybir.dt
    F32, B16 = dt.float32, dt.bfloat16
    RT = dt.float32r
    AF = mybir.ActivationFunctionType
    ALU = mybir.AluOpType
    AX = mybir.AxisListType

    # optimization knobs (env-tunable for debugging)
    use_pair = os.environ.get("KPAIR", "1") == "1"
    use_nomax = os.environ.get("KNOMAX", "1") == "1"
    ln1_trick = (os.environ.get("KTRICK", "1") == "1"
                 and not (flags["has_b1"] or flags["has_b2"] or flags["has_gb"]))
    pool_bcast = os.environ.get("KPOOLB", "0") == "1"

    nc = bass.Bass()

    # ---- DRAM I/O ----
    xw = nc.dram_tensor("xw", [BPC, 2, 128, D_BERT], RT, kind="ExternalInput")
    aw = nc.dram_tensor("aw", [BPC, 2, 128, T], RT, kind="ExternalInput")
    poh = nc.dram_tensor("poh", [32, BPC, T], RT, kind="ExternalInput")
    ptab = nc.dram_tensor("ptab", [32, D_POS], RT, kind="ExternalInput")
    wq32c = nc.dram_tensor("wq32c", [KD, 128, KD, 128], RT, kind="ExternalInput")
    wk32c = nc.dram_tensor("wk32c", [KD, 128, KD, 128], RT, kind="ExternalInput")
    wq16c = nc.dram_tensor("wq16c", [KD, 128, KD, 128], B16, kind="ExternalInput")
    wk16c = nc.dram_tensor("wk16c", [KD, 128, KD, 128], B16, kind="ExternalInput")
    wo16c = nc.dram_tensor("wo16c", [KD, 128, KD, 128], B16, kind="ExternalInput")
    w116c = nc.dram_tensor("w116c", [MF, 128, KD, 128], B16, kind="ExternalInput")
    w216c = nc.dram_tensor("w216c", [KD, 128, MF, 128], B16, kind="ExternalInput")
    wv16r = nc.dram_tensor("wv16r", [128, KD, D], B16, kind="ExternalInput")
    woutc = nc.dram_tensor("woutc", [NCH, 128, KD, 512], B16, kind="ExternalInput")
    biash = nc.dram_tensor("biash", [1, BPC * T], B16, kind="ExternalInput")
    id16 = nc.dram_tensor("id16", [128, 128], B16, kind="ExternalInput")
    id32 = nc.dram_tensor("id32", [128, 128], RT, kind="ExternalInput")
    # optional small params (always declared; tiny)
    bqkv32 = nc.dram_tensor("bqkv32", [1, 2 * D], F32, kind="ExternalInput")
    bsml16 = nc.dram_tensor("bsml16", [1, 6 * FF], B16, kind="ExternalInput")
    # rows: 0=bq',1=bk,2=bv,3=bo,4=b2,5=b1 (b1 uses full FF; others first D)
    bout16 = nc.dram_tensor("bout16", [1, NSP], B16, kind="ExternalInput")
    gb = nc.dram_tensor("gb", [4, D], F32, kind="ExternalInput")  # ln1g,ln1b,ln2g,ln2b
    out = nc.dram_tensor("out", [NTOK, NSENSE], F32, kind="ExternalOutput")

    with tile.TileContext(nc) as tc:
        cst = tc.tile_pool(name="cst", bufs=1)
        acts32 = tc.tile_pool(name="acts32", bufs=1)
        acts16 = tc.tile_pool(name="acts16", bufs=1)
        qkp = tc.tile_pool(name="qkp", bufs=2)
        vp = tc.tile_pool(name="vp", bufs=1)
        otp = tc.tile_pool(name="otp", bufs=1)
        ftp = tc.tile_pool(name="ftp", bufs=1)
        wbig = tc.tile_pool(name="wbig", bufs=1)
        wstr = tc.tile_pool(name="wstr", bufs=3)
        wqkp2 = tc.tile_pool(name="wqkp2", bufs=3)
        xap = tc.tile_pool(name="xap", bufs=2)
        plsb = tc.tile_pool(name="plsb", bufs=2)
        lnp = tc.tile_pool(name="lnp", bufs=2)
        lns = tc.tile_pool(name="lns", bufs=1)
        lnb = tc.tile_pool(name="lnb", bufs=1)
        sfp = tc.tile_pool(name="sfp", bufs=3)
        evp = tc.tile_pool(name="evp", bufs=2)
        psB = tc.tile_pool(name="psB", bufs=3, space="PSUM")
        psS = tc.tile_pool(name="psS", bufs=3, space="PSUM")
        psT = tc.tile_pool(name="psT", bufs=2, space="PSUM")
        ctxs = [cst, acts32, acts16, qkp, vp, otp, ftp, wbig, wstr, wqkp2,
                xap, plsb, lnp, lns, lnb, sfp, evp, psB, psS, psT]
        import contextlib
        with contextlib.ExitStack() as ctx:
            pools = [ctx.enter_context(p) for p in ctxs]
            (cst, acts32, acts16, qkp, vp, otp, ftp, wbig, wstr, wqkp2,
             xap, plsb, lnp, lns, lnb, sfp, evp, psB, psS, psT) = pools

            if pool_bcast:
                # partition_broadcast lives in the mlp gpsimd ucode library
                nc.gpsimd.load_library(library_config.mlp)

            # ---- pooling inputs first on the SP DMA queue ----
            xt_all, at_all = [], []
            for b in range(BPC):
                xt = []
                at = []
                for k in range(2):
                    x1 = xap.tile([128, D_BERT], RT, tag="x", bufs=2)
                    nc.sync.dma_start(x1[:], xw[b, k])
                    xt.append(x1)
                    a1 = xap.tile([128, T], RT, tag="a", bufs=2)
                    nc.sync.dma_start(a1[:], aw[b, k])
                    at.append(a1)
                xt_all.append(xt)
                at_all.append(at)
                if b == 0:
                    poh_sb = cst.tile([32, BPC, T], RT, tag="poh")
                    nc.sync.dma_start(poh_sb[:], poh[:])
                    ptab_sb = cst.tile([32, D_POS], RT, tag="ptab")
                    nc.sync.dma_start(ptab_sb[:], ptab[:])
                elif b == 1:
                    id32_sb = cst.tile([128, 128], RT, tag="id32")
                    nc.sync.dma_start(id32_sb[:], id32[:])
                    bh_sb = cst.tile([1, BPC * T], B16, tag="bh")
                    nc.sync.dma_start(bh_sb[:], biash[:])
                elif b == 2:
                    id_sb = cst.tile([128, 128], B16, tag="id")
                    nc.sync.dma_start(id_sb[:], id16[:])

            # V-projection weights on the Pool DMA queue, split in halves so
            # the first V matmuls can start early.
            wv_sb = wbig.tile([128, KD, D], B16, tag="wv")
            nc.gpsimd.dma_start(wv_sb[:, :, 0:512], wv16r[:, :, 0:512])
            nc.gpsimd.dma_start(wv_sb[:, :, 512:D], wv16r[:, :, 512:D])

            # ---- constants ----
            ones_r16 = cst.tile([1, 128], B16, tag="or16")
            nc.vector.memset(ones_r16[:], 1.0)
            ones_c16 = cst.tile([128, 1], B16, tag="oc16")
            nc.vector.memset(ones_c16[:], 1.0)
            any_bias = (flags["has_bqk"] or flags["has_bv"] or flags["has_bo"]
                        or flags["has_b1"] or flags["has_b2"])
            if any_bias:
                ones_r512_32 = cst.tile([1, 512], F32, tag="or512f")
                nc.vector.memset(ones_r512_32[:], 1.0)
                ones_r512_16 = cst.tile([1, 512], B16, tag="or512h")
                nc.vector.memset(ones_r512_16[:], 1.0)
            eps_sb = cst.tile([1, 1], F32, tag="eps")
            nc.vector.memset(eps_sb[:], 1e-5)
            if flags["has_bqk"]:
                bqkv_sb = cst.tile([1, 2 * D], F32, tag="bqkv")
                nc.sync.dma_start(bqkv_sb[:], bqkv32[:])
            if (flags["has_bv"] or flags["has_bo"] or flags["has_b1"]
                    or flags["has_b2"]):
                bsml_sb = cst.tile([1, 6 * FF], B16, tag="bsml")
                nc.sync.dma_start(bsml_sb[:], bsml16[:])
            if flags["has_bout"]:
                bout_sb = cst.tile([1, NSP], B16, tag="bout")
                nc.sync.dma_start(bout_sb[:], bout16[:])
            if flags["has_gb"]:
                gb_sb = cst.tile([4, D], F32, tag="gb")
                nc.sync.dma_start(gb_sb[:], gb[:])
                gbp = cst.tile([128, 4, KD], F32, tag="gbp")
                nc.sync.dma_start(
                    gbp[:], gb.rearrange("l (k p) -> p l k", p=128))

            # ---- phase 1: pooling (token-major) + V proj of layer 0 ----
            h32 = acts32.tile([128, KD, NTOK], RT, tag="a32")
            h16 = acts16.tile([128, KD, NTOK], B16, tag="a16", bufs=2)
            v16_l0 = vp.tile([128, BPC, D], B16, tag="v")

            def v_proj_b(b, v16, h16t):
                # V = h @ Wv for one sequence (token-major out)
                for n in range(2):
                    ps = psB.tile([128, 512], F32, tag="big")
                    nsl = slice(n * 512, (n + 1) * 512)
                    for k in range(KD):
                        nc.tensor.matmul(
                            ps[:], h16t[:, k, b * T:(b + 1) * T],
                            wv_sb[:, k, nsl], start=(k == 0),
                            stop=(k == KD - 1 and not flags["has_bv"]))
                    if flags["has_bv"]:
                        nc.tensor.matmul(
                            ps[:], ones_r16[:, :T],
                            bsml_sb[:, 2 * FF + n * 512:2 * FF + (n + 1) * 512],
                            start=False, stop=True)
                    if n == 0:
                        nc.vector.tensor_copy(v16[:, b, nsl], ps[:])
                    else:
                        nc.scalar.copy(v16[:, b, nsl], ps[:])

            for b in range(BPC):
                xt, at = xt_all[b], at_all[b]
                bsl = slice(b * T, (b + 1) * T)
                psA_ = psS.tile([128, 512], F32, tag="ss")
                for k in range(2):
                    nc.tensor.matmul(psA_[:], at[k][:], xt[k][:, 0:512],
                                     start=(k == 0), stop=(k == 1))
                psB_ = psS.tile([128, 512], F32, tag="ss")
                for k in range(2):
                    nc.tensor.matmul(psB_[:, 0:256], at[k][:], xt[k][:, 512:D_BERT],
                                     start=(k == 0), stop=(k == 1))
                nc.tensor.matmul(psB_[:, 256:512], poh_sb[:, b, :], ptab_sb[:],
                                 start=True, stop=True)
                sA = plsb.tile([128, 512], RT, tag="pl", bufs=3)
                nc.vector.tensor_copy(sA[:], psA_[:])
                sB = plsb.tile([128, 512], RT, tag="pl", bufs=3)
                nc.scalar.copy(sB[:], psB_[:])
                trA = psB.tile([128, 4, 128], RT, tag="big")
                for mm in range(4):
                    nc.tensor.transpose(trA[:, mm, :],
                                        sA[:, mm * 128:(mm + 1) * 128], id32_sb[:])
                trB = psB.tile([128, 4, 128], RT, tag="big")
                for mm in range(4):
                    nc.tensor.transpose(trB[:, mm, :],
                                        sB[:, mm * 128:(mm + 1) * 128], id32_sb[:])
                nc.vector.tensor_copy(h32[:, 0:4, bsl], trA[:])
                nc.scalar.copy(h16[:, 0:4, bsl], trA[:])
                nc.vector.tensor_copy(h32[:, 4:8, bsl], trB[:])
                nc.scalar.copy(h16[:, 4:8, bsl], trB[:])
                v_proj_b(b, v16_l0, h16)

            # ---- transformer layers ----
            for li in range(NL):
                first = (li == 0)
                dtq = RT if first else B16
                h_rhs = h32 if first else h16
                wq_d, wk_d = (wq32c, wk32c) if first else (wq16c, wk16c)

                if first:
                    v16 = v16_l0
                else:
                    v16 = vp.tile([128, BPC, D], B16, tag="v")
                    for b in range(BPC):
                        v_proj_b(b, v16, h16)

                ot16 = otp.tile([128, KD, NTOK], B16, tag="ot")
                qkt = {}

                def qkproj(m):
                    wqc = wqkp2.tile([128, KD, 128], dtq, tag="wqk")
                    nc.gpsimd.dma_start(wqc[:], wq_d[m])
                    wkc = wqkp2.tile([128, KD, 128], dtq, tag="wqk")
                    nc.gpsimd.dma_start(wkc[:], wk_d[m])
                    qt = qkp.tile([128, NTOK], dtq, tag="q")
                    kt = qkp.tile([128, NTOK], dtq, tag="k")
                    for dst, wc, brow, on_dve in ((qt, wqc, 0, True),
                                                  (kt, wkc, 1, False)):
                        ps = psB.tile([128, 512], F32, tag="big")
                        for k in range(KD):
                            nc.tensor.matmul(
                                ps[:], wc[:, k, :], h_rhs[:, k, :],
                                start=(k == 0),
                                stop=(k == KD - 1 and not flags["has_bqk"]))
                        if flags["has_bqk"]:
                            nc.tensor.matmul(
                                ps[:], bqkv_sb[:, brow * D + m * 128:
                                               brow * D + (m + 1) * 128],
                                ones_r512_32[:], start=False, stop=True)
                        if on_dve:
                            nc.vector.tensor_copy(dst[:], ps[:])
                        else:
                            nc.scalar.copy(dst[:], ps[:])
                    qkt[m] = (qt, kt)

                qkproj(0)
                for m in range(KD):
                    qt, kt = qkt.pop(m)
                    ops = psB.tile([128, 512], F32, tag="big")
                    sstiles = []
                    for h2 in (0, 1):
                        hsl = slice(64 * h2, 64 * h2 + 64)
                        prs = []
                        for p in (0, 1):
                            sst = psS.tile(
                                [128, 2, 256 if (first and use_pair) else T],
                                F32, tag="ss")
                            for j in (0, 1):
                                b = 2 * p + j
                                bsl = slice(b * T, (b + 1) * T)
                                if first and use_pair:
                                    ksl = slice(p * 256, p * 256 + 256)
                                    nc.tensor.matmul(sst[:, j, :],
                                                     qt[hsl, bsl], kt[hsl, ksl],
                                                     start=True, stop=False)
                                    off = 128 * j
                                    nc.tensor.matmul(
                                        sst[:, j, off:off + 128], ones_r16[:],
                                        bh_sb[:, bsl], start=False, stop=True)
                                else:
                                    nc.tensor.matmul(sst[:, j, :],
                                                     qt[hsl, bsl], kt[hsl, bsl],
                                                     start=True, stop=False)
                                    nc.tensor.matmul(
                                        sst[:, j, :], ones_r16[:],
                                        bh_sb[:, bsl], start=False, stop=True)
                            prs.append(sst)
                        sstiles.append(prs)
                    if m + 1 < KD:
                        qkproj(m + 1)
                    ptps = []
                    for h2 in (0, 1):
                        prs = sstiles[h2]
                        ex = sfp.tile([128, BPC, T], B16, tag="ex")
                        den4 = sfp.tile([128, BPC], F32, tag="den")
                        skip_max = (not first) and use_nomax
                        if skip_max:
                            for p in (0, 1):
                                for j in (0, 1):
                                    b = 2 * p + j
                                    src = (prs[p][:, j, 128 * j:128 * j + 128]
                                           if (first and use_pair)
                                           else prs[p][:, j, :])
                                    nc.scalar.activation(
                                        ex[:, b, :], src, AF.Exp,
                                        accum_out=den4[:, b:b + 1])
                        else:
                            nm4 = sfp.tile([128, BPC], F32, tag="nm")
                            for p in (0, 1):
                                for j in (0, 1):
                                    b = 2 * p + j
                                    src = (prs[p][:, j, 128 * j:128 * j + 128]
                                           if (first and use_pair)
                                           else prs[p][:, j, :])
                                    nc.vector.tensor_reduce(
                                        nm4[:, b:b + 1], src, axis=AX.X,
                                        op=ALU.max, negate=True)
                            for p in (0, 1):
                                for j in (0, 1):
                                    b = 2 * p + j
                                    src = (prs[p][:, j, 128 * j:128 * j + 128]
                                           if (first and use_pair)
                                           else prs[p][:, j, :])
                                    nc.scalar.activation(
                                        ex[:, b, :], src, AF.Exp,
                                        bias=nm4[:, b:b + 1], scale=1.0,
                                        accum_out=den4[:, b:b + 1])
                        rcp4 = sfp.tile([128, BPC], F32, tag="rcp")
                        nc.vector.reciprocal(rcp4[:], den4[:])
                        pn = sfp.tile([128, BPC, T], B16, tag="pn")
                        nc.vector.tensor_tensor(
                            pn[:], ex[:],
                            rcp4[:, :, None].to_broadcast((128, BPC, T)),
                            ALU.mult)
                        ptp = psT.tile([128, BPC, T], B16, tag="pt")
                        for b in range(BPC):
                            nc.tensor.transpose(ptp[:, b, :], pn[:, b, :],
                                                id_sb[:])
                        ptps.append(ptp)
                    pts_l = []
                    for h2 in (0, 1):
                        pts = sfp.tile([128, BPC, T], B16, tag="pts")
                        nc.scalar.copy(pts[:], ptps[h2][:])
                        pts_l.append(pts)
                    for h2 in (0, 1):
                        hsl = slice(64 * h2, 64 * h2 + 64)
                        head = 2 * m + h2
                        for b in range(BPC):
                            nc.tensor.matmul(
                                ops[hsl, b * T:(b + 1) * T],
                                v16[:, b, head * 64:(head + 1) * 64],
                                pts_l[h2][:, b, :], start=True, stop=True)
                    nc.vector.tensor_copy(ot16[:, m, :], ops[:])

                # O projection + residual (bf16) + LN1 sum accumulation
                r16 = acts16.tile([128, KD, NTOK], B16, tag="r", bufs=1)
                ps1a = psT.tile([1, 512], F32, tag="pt")
                if not ln1_trick:
                    ps2a = psT.tile([1, 512], F32, tag="pt")
                for m in range(KD):
                    woc = wstr.tile([128, KD, 128], B16, tag="w")
                    nc.gpsimd.dma_start(woc[:], wo16c[m])
                    ps = psB.tile([128, 512], F32, tag="big")
                    for k in range(KD):
                        nc.tensor.matmul(
                            ps[:], woc[:, k, :], ot16[:, k, :],
                            start=(k == 0),
                            stop=(k == KD - 1 and not flags["has_bo"]))
                    if flags["has_bo"]:
                        nc.tensor.matmul(
                            ps[:], bsml_sb[:, 3 * FF + m * 128:3 * FF + (m + 1) * 128],
                            ones_r512_16[:], start=False, stop=True)
                    nc.vector.tensor_tensor(r16[:, m, :], ps[:],
                                            h16[:, m, :], ALU.add)
                    nc.tensor.matmul(ps1a[:], ones_c16[:], r16[:, m, :],
                                     start=(m == 0), stop=(m == KD - 1))
                    if not ln1_trick:
                        sq = lnp.tile([128, 512], B16, tag="sq")
                        nc.vector.tensor_mul(sq[:], r16[:, m, :], r16[:, m, :])
                        nc.tensor.matmul(ps2a[:], ones_c16[:], sq[:],
                                         start=(m == 0), stop=(m == KD - 1))

                def bcast16(dst, src):
                    # [1,512] bf16 -> [128,512] bf16 broadcast
                    if pool_bcast:
                        nc.gpsimd.partition_broadcast(dst[:], src[:])
                    else:
                        bps = psB.tile([128, 512], F32, tag="big")
                        nc.tensor.matmul(bps[:], ones_r16[:], src[:],
                                         start=True, stop=True)
                        nc.scalar.copy(dst[:], bps[:])

                def ln_tail(ps1, ps2, vin16, g_idx, b_idx, out_tag):
                    """Full LN: vin16 (bf16 residual) -> normalized bf16."""
                    mu = lns.tile([1, 512], F32, tag="mu")
                    nc.vector.tensor_scalar_mul(mu[:], ps1[:], 1.0 / D)
                    m2 = lns.tile([1, 512], F32, tag="m2")
                    nc.vector.tensor_mul(m2[:], mu[:], mu[:])
                    tmp = lns.tile([1, 512], F32, tag="tmp")
                    nc.vector.scalar_tensor_tensor(
                        tmp[:], ps2[:], 1.0 / D, m2[:],
                        ALU.mult, ALU.subtract)
                    nc.scalar.activation(tmp[:], tmp[:], AF.Ln,
                                         bias=eps_sb[:], scale=1.0)
                    rstd = lns.tile([1, 512], B16, tag="rstd")
                    nc.scalar.activation(rstd[:], tmp[:], AF.Exp, scale=-0.5)
                    mrs = lns.tile([1, 512], B16, tag="mrs")
                    nc.vector.scalar_tensor_tensor(
                        mrs[:], mu[:], -1.0, rstd[:], ALU.mult, ALU.mult)
                    rsb = lnb.tile([128, 512], B16, tag="rsb")
                    bcast16(rsb, rstd)
                    mrsb = lnb.tile([128, 512], B16, tag="mrsb")
                    bcast16(mrsb, mrs)
                    o16 = acts16.tile([128, KD, NTOK], B16, tag=out_tag,
                                      bufs=2 if out_tag == "a16" else 1)
                    for k in range(KD):
                        t = lnp.tile([128, 512], B16, tag="t")
                        nc.vector.tensor_mul(t[:], vin16[:, k, :], rsb[:])
                        nc.vector.tensor_tensor(o16[:, k, :], t[:], mrsb[:],
                                                ALU.add)
                        if flags["has_gb"]:
                            nc.vector.tensor_scalar(
                                o16[:, k, :], o16[:, k, :],
                                gbp[:, g_idx, k:k + 1], gbp[:, b_idx, k:k + 1],
                                ALU.mult, ALU.add)
                    return o16

                if ln1_trick:
                    # LN1 reduces to mean-centering (scale/shift cancel in LN2)
                    mu = lns.tile([1, 512], F32, tag="mu")
                    nc.vector.tensor_scalar_mul(mu[:], ps1a[:], 1.0 / D)
                    mu16 = lns.tile([1, 512], B16, tag="mu16")
                    nc.scalar.copy(mu16[:], mu[:])
                    mub = lnb.tile([128, 512], B16, tag="mub")
                    bcast16(mub, mu16)
                    h1_16 = acts16.tile([128, KD, NTOK], B16, tag="h1")
                    for k in range(KD):
                        nc.vector.tensor_sub(h1_16[:, k, :], r16[:, k, :],
                                             mub[:])
                else:
                    h1_16 = ln_tail(ps1a, ps2a, r16, 0, 1, "h1")

                # FFN
                ft = ftp.tile([128, MF, NTOK], B16, tag="ft")
                for mf in range(MF):
                    w1c = wstr.tile([128, KD, 128], B16, tag="w")
                    nc.gpsimd.dma_start(w1c[:], w116c[mf])
                    ps = psB.tile([128, 512], F32, tag="big")
                    for k in range(KD):
                        nc.tensor.matmul(
                            ps[:], w1c[:, k, :], h1_16[:, k, :],
                            start=(k == 0),
                            stop=(k == KD - 1 and not flags["has_b1"]))
                    if flags["has_b1"]:
                        nc.tensor.matmul(
                            ps[:], bsml_sb[:, 5 * FF + mf * 128:5 * FF + (mf + 1) * 128],
                            ones_r512_16[:], start=False, stop=True)
                    nc.scalar.activation(ft[:, mf, :], ps[:], AF.Relu)
                r2 = acts16.tile([128, KD, NTOK], B16, tag="r", bufs=1)
                ps1b = psT.tile([1, 512], F32, tag="pt")
                ps2b = psT.tile([1, 512], F32, tag="pt")
                for m in range(KD):
                    w2c = wstr.tile([128, MF, 128], B16, tag="w")
                    nc.gpsimd.dma_start(w2c[:], w216c[m])
                    ps = psB.tile([128, 512], F32, tag="big")
                    for k in range(MF):
                        nc.tensor.matmul(
                            ps[:], w2c[:, k, :], ft[:, k, :],
                            start=(k == 0),
                            stop=(k == MF - 1 and not flags["has_b2"]))
                    if flags["has_b2"]:
                        nc.tensor.matmul(
                            ps[:], bsml_sb[:, 4 * FF + m * 128:4 * FF + (m + 1) * 128],
                            ones_r512_16[:], start=False, stop=True)
                    nc.vector.tensor_tensor(r2[:, m, :], ps[:],
                                            h1_16[:, m, :], ALU.add)
                    nc.tensor.matmul(ps1b[:], ones_c16[:], r2[:, m, :],
                                     start=(m == 0), stop=(m == KD - 1))
                    sq = lnp.tile([128, 512], B16, tag="sq")
                    nc.vector.tensor_mul(sq[:], r2[:, m, :], r2[:, m, :])
                    nc.tensor.matmul(ps2b[:], ones_c16[:], sq[:],
                                     start=(m == 0), stop=(m == KD - 1))

                h16 = ln_tail(ps1b, ps2b, r2, 2, 3, "a16")

            # ---- final vocab projection (token-major) ----
            for n in range(NCH):
                ncols = 512 if n < NCH - 1 else NSENSE - 512 * (NCH - 1)
                woc = wstr.tile([128, KD, 512], B16, tag="w")
                nc.gpsimd.dma_start(woc[:, :, :ncols], woutc[n][:, :, :ncols])
                for mt in range(BPC):
                    tsl = slice(mt * 128, (mt + 1) * 128)
                    ps = psB.tile([128, 512], F32, tag="big")
                    for k in range(KD):
                        nc.tensor.matmul(
                            ps[:, :ncols], h16[:, k, tsl], woc[:, k, :ncols],
                            start=(k == 0),
                            stop=(k == KD - 1 and not flags["has_bout"]))
                    if flags["has_bout"]:
                        nc.tensor.matmul(
                            ps[:, :ncols], ones_r16[:],
                            bout_sb[:, n * 512:n * 512 + ncols],
                            start=False, stop=True)
                    lg = evp.tile([128, 512], F32, tag="lg")
                    if mt % 2 == 0:
                        nc.scalar.copy(lg[:, :ncols], ps[:, :ncols])
                    else:
                        nc.vector.tensor_copy(lg[:, :ncols], ps[:, :ncols])
                    nc.sync.dma_start(out[tsl, n * 512:n * 512 + ncols],
                                      lg[:, :ncols])

    _split_multi_waits(nc)
    nc.finalize()
    return nc


# ---------------------------------------------------------------------------
# Host-side prep + run
# ---------------------------------------------------------------------------

def _prep(inputs):
    """Build per-core in_maps from full inputs."""
    x = np.asarray(inputs["x"], np.float32)
    word_ids = np.asarray(inputs["word_ids"], np.int32)
    text_lengths = np.asarray(inputs["text_lengths"], np.int32)
    pos_tags = np.asarray(inputs["pos_tags"], np.int64)
    pos_table = np.asarray(inputs["pos_table"], np.float32)

    # pooling matrix A[b, s, t] = SCALE / cnt[b, t] if word_ids[b,s]==t
    cnt = np.zeros((B, T), np.float32)
    np.add.at(cnt, (np.arange(B)[:, None], word_ids), 1.0)
    cntc = np.maximum(cnt, 1.0)
    A = np.zeros((B, S, T), np.float32)
    bi = np.repeat(np.arange(B), S)
    si = np.tile(np.arange(S), B)
    ti = word_ids.ravel()
    A[bi, si, ti] = SCALE / cntc[bi, ti]

    # pos one-hot (padded to 32 rows) x SCALE
    poh = np.zeros((B, 32, T), np.float32)
    poh[np.repeat(np.arange(B), T), pos_tags.ravel().astype(np.int64),
        np.tile(np.arange(T), B)] = SCALE
    ptab = np.zeros((32, D_POS), np.float32)
    ptab[:NPOS] = pos_table

    key_mask = np.arange(T)[None, :] < text_lengths[:, None]
    bias_row = np.where(key_mask, 0.0, -1e9).astype(np.float32)

    Wqs = (np.asarray(inputs["Wq"], np.float32) * ATTN_SCALE)
    Wk = np.asarray(inputs["Wk"], np.float32)
    Wv = np.asarray(inputs["Wv"], np.float32)
    Wo = np.asarray(inputs["Wo"], np.float32)
    W1 = np.asarray(inputs["W1"], np.float32)
    W2 = np.asarray(inputs["W2"], np.float32)
    Wout = np.asarray(inputs["Wout"], np.float32)

    def colchunk(w, asdt):
        din, dout = w.shape
        return np.ascontiguousarray(
            w.reshape(din // 128, 128, dout // 128, 128).transpose(2, 1, 0, 3)
        ).astype(asdt)

    wq32c = colchunk(Wqs, np.float32)
    wk32c = colchunk(Wk, np.float32)
    wq16c = colchunk(Wqs, BF16)
    wk16c = colchunk(Wk, BF16)
    wo16c = colchunk(Wo, BF16)
    w116c = colchunk(W1, BF16)
    w216c = colchunk(W2, BF16)
    wv16r = np.ascontiguousarray(
        Wv.reshape(KD, 128, D).transpose(1, 0, 2)).astype(BF16)
    Wout_p = np.zeros((D, NSP), np.float32)
    Wout_p[:, :NSENSE] = Wout
    woutc = np.ascontiguousarray(
        Wout_p.reshape(KD, 128, NCH, 512).transpose(2, 1, 0, 3)).astype(BF16)

    bq = np.asarray(inputs["bq"], np.float32) * ATTN_SCALE
    bk = np.asarray(inputs["bk"], np.float32)
    bqkv32 = np.stack([bq, bk]).astype(np.float32).reshape(1, 2 * D)
    bsml = np.zeros((6, FF), np.float32)
    bsml[2, :D] = np.asarray(inputs["bv"], np.float32)
    bsml[3, :D] = np.asarray(inputs["bo"], np.float32)
    bsml[4, :D] = np.asarray(inputs["b2"], np.float32)
    bsml[5] = np.asarray(inputs["b1"], np.float32)
    bout = np.zeros((1, NSP), np.float32)
    bout[0, :NSENSE] = np.asarray(inputs["bout"], np.float32)
    gbarr = np.stack([np.asarray(inputs["ln1_g"], np.float32),
                      np.asarray(inputs["ln1_b"], np.float32),
                      np.asarray(inputs["ln2_g"], np.float32),
                      np.asarray(inputs["ln2_b"], np.float32)])

    flags = {
        "has_bqk": bool(np.any(bqkv32)),
        "has_bv": bool(np.any(bsml[2])),
        "has_bo": bool(np.any(bsml[3])),
        "has_b2": bool(np.any(bsml[4])),
        "has_b1": bool(np.any(bsml[5])),
        "has_bout": bool(np.any(bout)),
        "has_gb": bool(np.any(gbarr[1]) or np.any(gbarr[3])
                       or not np.all(gbarr[0] == 1.0)
                       or not np.all(gbarr[2] == 1.0)),
    }

    ident16 = np.eye(128, dtype=np.float32).astype(BF16)
    ident32 = np.eye(128, dtype=np.float32)

    shared = dict(
        wq32c=wq32c, wk32c=wk32c, wq16c=wq16c, wk16c=wk16c, wo16c=wo16c,
        w116c=w116c, w216c=w216c, wv16r=wv16r, woutc=woutc, ptab=ptab,
        id16=ident16, id32=ident32, bqkv32=bqkv32,
        bsml16=bsml.astype(BF16).reshape(1, 6 * FF),
        bout16=bout.astype(BF16), gb=gbarr,
    )

    in_maps = []
    for c in range(NCORES):
        bsl = slice(c * BPC, (c + 1) * BPC)
        m = dict(shared)
        m["xw"] = np.ascontiguousarray(
            x[bsl].reshape(BPC, 2, 128, D_BERT))
        m["aw"] = np.ascontiguousarray(A[bsl].reshape(BPC, 2, 128, T))
        m["poh"] = np.ascontiguousarray(poh[bsl].transpose(1, 0, 2))
        m["biash"] = np.ascontiguousarray(bias_row[bsl]).reshape(1, BPC * T).astype(BF16)
        in_maps.append(m)
    return in_maps, flags


def kernel(**inputs) -> np.ndarray:
    in_maps, flags = _prep(inputs)
    key = (os.environ.get("KPAIR", "1"), os.environ.get("KNOMAX", "1"),
           os.environ.get("KTRICK", "1"), os.environ.get("KPOOLB", "0"),
           ) + tuple(sorted(flags.items()))
    if key not in _BUILD_CACHE:
        _BUILD_CACHE[key] = _build(flags)
    nc = _BUILD_CACHE[key]

    if os.environ.get("KERNEL_SIM") == "1":
        from concourse.bass_interp import CoreSim
        ncore = int(os.environ.get("KERNEL_SIM_CORES", "1"))
        outs = []
        for c in range(ncore):
            sim = CoreSim(nc)
            for name, arr in in_maps[c].items():
                sim.tensor(name)[:] = arr
            sim.simulate()
            outs.append(np.asarray(sim.tensor("out")).copy())
        full = np.zeros((B, T, NSENSE), np.float32)
        for c in range(ncore):
            full[c * BPC:(c + 1) * BPC] = outs[c].reshape(BPC, T, NSENSE)
        return full

    from concourse.bass_utils import run_bass_kernel_spmd
    r = run_bass_kernel_spmd(nc, in_maps, core_ids=list(range(NCORES)))
    full = np.concatenate(
        [r.results[c]["out"].reshape(BPC, T, NSENSE) for c in range(NCORES)],
        axis=0)
    return full


# revision 49
# speedup vs baseline: 2.1000x; 2.1000x over previous
"""Trainium2 Bass kernel for nn_BertTransformerWSD.

Takes FULL inputs, shards batch over 8 NeuronCores (4 sequences/core),
runs a fused transformer kernel per core, gathers full output.

Numerics: bf16 matmuls with fp32 PSUM accumulate everywhere except the
precision-critical path (segment-mean pooling and layer-1 Q/K
projections + QK^T run in fp32r) -- layer-1 attention logits are
O(+-600) and softmax is near-argmax, so S needs absolute accuracy ~0.1.

Structure (per core, 4 seqs x 128 words = 512 tokens):
- pooling: token-major matmuls (A^T stationary, x moving; f32r at
  >=256-wide free dim runs at full PE rate), transpose back to
  feature-major h32 (f32r) + h16 (bf16); V-projection of layer 0 is
  interleaved into the pooling loop per seq.
- attention: layer-0 QK^T is pair-batched over 2 seqs (256-wide moving
  window) so f32r runs at 1 cyc/row; key-mask bias added via small bf16
  matmuls; softmax = DVE max (layer 0 only) -> Act exp with
  per-partition bias and accum_out denominators -> DVE reciprocal +
  normalize; PE transposes P, then AV.
- LN1 is reduced to mean-centering only: LayerNorm is invariant to a
  per-token affine and ReLU is positively homogeneous, so the rstd
  scale cancels inside LN2 (valid because b1=b2=0 and ln affine is
  identity for this problem; falls back to full LN otherwise).
- LN2: feature sums accumulated on PE while Wo/FFN2 still run; Pool
  engine broadcasts rstd/-mu*rstd; bf16 normalize on DVE.
- weight DMAs issued from the (otherwise idle) Pool engine queue,
  input/output DMAs from SP, to avoid single-queue serialization.
- vocab projection: exact 5000 columns (last chunk 392 wide).
"""
import os
import numpy as np
import ml_dtypes

# ---- model constants (hardcoded; must match reference.py) ----
B, S, T = 32, 256, 128
D_BERT, D_POS, D = 768, 256, 1024
H, DH, FF = 16, 64, 4096
NL = 2
NSENSE, NPOS = 5000, 20
SCALE = float(np.sqrt(D))
ATTN_SCALE = 1.0 / float(np.sqrt(DH))

NCORES = 8
BPC = B // NCORES           # sequences per core = 4
NTOK = BPC * T              # tokens per core = 512
KD = D // 128               # 8 k-tiles over D
MF = FF // 128              # 32 m-chunks over FF
NSP = 5120                  # padded NSENSE (10 x 512)
NCH = NSP // 512            # 10 sense chunks

BF16 = ml_dtypes.bfloat16

_BUILD_CACHE = {}


# ---------------------------------------------------------------------------
# Tile/walrus compatibility patches
# ---------------------------------------------------------------------------

def _install_patches():
    import concourse.mybir as mybir
    import concourse.tile as tile

    if getattr(tile.TileContext, "_wsd_patched", False):
        return

    def _patched_drain_and_barrier(self, tick_clock, wait_clock):
        # walrus in this container accepts at most ONE sem wait per
        # instruction; the stock exit drain carries one wait per active
        # logical processor.  Split them across SP nops.
        from concourse.tile import ScopedClock
        nc = self.nc
        probe = nc.sync.nop()
        wait_clock.add_sem_waits(probe.ins,
                                 ScopedClock({None: tick_clock.global_clock}))
        si = probe.ins.sync_info
        waits = list(si.on_wait) if si is not None and si.on_wait else []
        if len(waits) > 1:
            probe.ins.sync_info = mybir.SyncInfo(on_wait=waits[:1], on_update=[])
            for w in waits[1:]:
                n2 = nc.sync.nop()
                n2.ins.sync_info = mybir.SyncInfo(on_wait=[w], on_update=[])
        nc.sync.drain()
        nc.all_engine_barrier()
        assert self.sems is not None
        popped = nc._tile_sem_poison_stack.pop()
        assert popped is self._sem_poison
        nc.clear_and_free_semaphores(list(self.sems.allocated().values()))
        nc.all_engine_barrier()

    tile.TileContext._drain_and_barrier = _patched_drain_and_barrier
    tile.TileContext._wsd_patched = True


def _split_multi_waits(nc):
    """Safety net: split any instruction carrying >1 sem waits into
    engine-matched NoOps (sequential waits == one multi-wait)."""
    import concourse.mybir as mybir
    n = 0
    for func in nc.m.functions:
        for blk in func.blocks:
            insts = list(blk.instructions)
            rebuilt = []
            changed = False
            for inst in insts:
                si = inst.sync_info
                waits = list(si.on_wait) if si is not None and si.on_wait else []
                if len(waits) > 1:
                    for w in waits[:-1]:
                        nop = mybir.InstNoOp(name=f"I-wsplit-{n}", ins=[], outs=[])
                        n += 1
                        nop.engine = inst.engine
                        nop.sync_info = mybir.SyncInfo(on_wait=[w], on_update=[])
                        nc.register_instruction(nop, overwrite=True)
                        rebuilt.append(nop)
                    inst.sync_info = mybir.SyncInfo(
                        on_wait=[waits[-1]],
                        on_update=list(si.on_update) if si.on_update else [])
                    changed = True
                rebuilt.append(inst)
            if changed:
                while len(blk.instructions):
                    blk.instructions.pop()
                for i in rebuilt:
                    blk.instructions.append(i)


# ---------------------------------------------------------------------------
# Device kernel build
# ---------------------------------------------------------------------------

def _build(flags):
    """flags: dict of bools {has_bqk, has_bv, ..., has_gb}."""
    import concourse.bass as bass
    import concourse.mybir as mybir
    import concourse.tile as tile
    from concourse import library_config

    _install_patches()
    dt = mybir.dt
    F32, B16 = dt.float32, dt.bfloat16
    RT = dt.float32r
    AF = mybir.ActivationFunctionType
    ALU = mybir.AluOpType
    AX = mybir.AxisListType

    # optimization knobs (env-tunable for debugging)
    use_pair = os.environ.get("KPAIR", "1") == "1"
    use_nomax = os.environ.get("KNOMAX", "1") == "1"
    ln1_trick = (os.environ.get("KTRICK", "1") == "1"
                 and not (flags["has_b1"] or flags["has_b2"] or flags["has_gb"]))
    pool_bcast = os.environ.get("KPOOLB", "0") == "1"

    nc = bass.Bass()

    # ---- DRAM I/O ----
    xw = nc.dram_tensor("xw", [BPC, 2, 128, D_BERT], RT, kind="ExternalInput")
    aw = nc.dram_tensor("aw", [BPC, 2, 128, T], RT, kind="ExternalInput")
    poh = nc.dram_tensor("poh", [32, BPC, T], RT, kind="ExternalInput")
    ptab = nc.dram_tensor("ptab", [32, D_POS], RT, kind="ExternalInput")
    wq32c = nc.dram_tensor("wq32c", [KD, 128, KD, 128], RT, kind="ExternalInput")
    wk32c = nc.dram_tensor("wk32c", [KD, 128, KD, 128], RT, kind="ExternalInput")
    wq16c = nc.dram_tensor("wq16c", [KD, 128, KD, 128], B16, kind="ExternalInput")
    wk16c = nc.dram_tensor("wk16c", [KD, 128, KD, 128], B16, kind="ExternalInput")
    wo16c = nc.dram_tensor("wo16c", [KD, 128, KD, 128], B16, kind="ExternalInput")
    w116c = nc.dram_tensor("w116c", [MF, 128, KD, 128], B16, kind="ExternalInput")
    w216c = nc.dram_tensor("w216c", [KD, 128, MF, 128], B16, kind="ExternalInput")
    wv16r = nc.dram_tensor("wv16r", [128, KD, D], B16, kind="ExternalInput")
    woutc = nc.dram_tensor("woutc", [NCH, 128, KD, 512], B16, kind="ExternalInput")
    biash = nc.dram_tensor("biash", [1, BPC * T], B16, kind="ExternalInput")
    id16 = nc.dram_tensor("id16", [128, 128], B16, kind="ExternalInput")
    id32 = nc.dram_tensor("id32", [128, 128], RT, kind="ExternalInput")
    # optional small params (always declared; tiny)
    bqkv32 = nc.dram_tensor("bqkv32", [1, 2 * D], F32, kind="ExternalInput")
    bsml16 = nc.dram_tensor("bsml16", [1, 6 * FF], B16, kind="ExternalInput")
    # rows: 0=bq',1=bk,2=bv,3=bo,4=b2,5=b1 (b1 uses full FF; others first D)
    bout16 = nc.dram_tensor("bout16", [1, NSP], B16, kind="ExternalInput")
    gb = nc.dram_tensor("gb", [4, D], F32, kind="ExternalInput")  # ln1g,ln1b,ln2g,ln2b
    out = nc.dram_tensor("out", [NTOK, NSENSE], F32, kind="ExternalOutput")

    with tile.TileContext(nc) as tc:
        cst = tc.tile_pool(name="cst", bufs=1)
        acts32 = tc.tile_pool(name="acts32", bufs=1)
        acts16 = tc.tile_pool(name="acts16", bufs=1)
        qkp = tc.tile_pool(name="qkp", bufs=2)
        vp = tc.tile_pool(name="vp", bufs=1)
        otp = tc.tile_pool(name="otp", bufs=1)
        ftp = tc.tile_pool(name="ftp", bufs=1)
        wbig = tc.tile_pool(name="wbig", bufs=1)
        wstr = tc.tile_pool(name="wstr", bufs=3)
        wqkp2 = tc.tile_pool(name="wqkp2", bufs=3)
        xap = tc.tile_pool(name="xap", bufs=2)
        plsb = tc.tile_pool(name="plsb", bufs=2)
        lnp = tc.tile_pool(name="lnp", bufs=2)
        lns = tc.tile_pool(name="lns", bufs=1)
        lnb = tc.tile_pool(name="lnb", bufs=1)
        sfp = tc.tile_pool(name="sfp", bufs=3)
        evp = tc.tile_pool(name="evp", bufs=2)
        psB = tc.tile_pool(name="psB", bufs=3, space="PSUM")
        psS = tc.tile_pool(name="psS", bufs=3, space="PSUM")
        psT = tc.tile_pool(name="psT", bufs=2, space="PSUM")
        ctxs = [cst, acts32, acts16, qkp, vp, otp, ftp, wbig, wstr, wqkp2,
                xap, plsb, lnp, lns, lnb, sfp, evp, psB, psS, psT]
        import contextlib
        with contextlib.ExitStack() as ctx:
            pools = [ctx.enter_context(p) for p in ctxs]
            (cst, acts32, acts16, qkp, vp, otp, ftp, wbig, wstr, wqkp2,
             xap, plsb, lnp, lns, lnb, sfp, evp, psB, psS, psT) = pools

            if pool_bcast:
                # partition_broadcast lives in the mlp gpsimd ucode library
                nc.gpsimd.load_library(library_config.mlp)

            # ---- pooling inputs: A-halves (feats 0:512) on SP, B-halves on
            # Act queue, so the first pool matmul can start ~1us earlier and
            # the two streams run in parallel.  b0's DMAs are emitted here;
            # later b's are prefetched from inside the pooling loop.
            def emit_input_dmas(b):
                xt = []
                for k in range(2):
                    a1 = xap.tile([128, T], RT, tag="a", bufs=2)
                    nc.sync.dma_start(a1[:], aw[b, k])
                    x1 = xap.tile([128, D_BERT], RT, tag="x", bufs=2)
                    nc.sync.dma_start(x1[:, 0:512], xw[b, k][:, 0:512])
                    nc.scalar.dma_start(x1[:, 512:D_BERT],
                                        xw[b, k][:, 512:D_BERT])
                    xt.append((x1, a1))
                return xt

            xa_all = [None] * BPC
            xa_all[0] = emit_input_dmas(0)
            ptab_sb = cst.tile([32, D_POS], RT, tag="ptab")
            nc.scalar.dma_start(ptab_sb[:], ptab[:])
            poh_sb = cst.tile([32, BPC, T], RT, tag="poh")
            nc.scalar.dma_start(poh_sb[:], poh[:])
            id32_sb = cst.tile([128, 128], RT, tag="id32")
            nc.sync.dma_start(id32_sb[:], id32[:])
            bh_sb = cst.tile([1, BPC * T], B16, tag="bh")
            nc.scalar.dma_start(bh_sb[:], biash[:])
            id_sb = cst.tile([128, 128], B16, tag="id")
            nc.scalar.dma_start(id_sb[:], id16[:])

            # V-projection weights on the Pool DMA queue, split in halves so
            # the first V matmuls can start early.
            wv_sb = wbig.tile([128, KD, D], B16, tag="wv")
            nc.gpsimd.dma_start(wv_sb[:, 0:4, 0:512], wv16r[:, 0:4, 0:512])
            nc.gpsimd.dma_start(wv_sb[:, 4:KD, 0:512], wv16r[:, 4:KD, 0:512])
            nc.gpsimd.dma_start(wv_sb[:, 0:4, 512:D], wv16r[:, 0:4, 512:D])
            nc.gpsimd.dma_start(wv_sb[:, 4:KD, 512:D], wv16r[:, 4:KD, 512:D])

            # ---- constants ----
            ones_r16 = cst.tile([1, 128], B16, tag="or16")
            nc.vector.memset(ones_r16[:], 1.0)
            ones_c16 = cst.tile([128, 1], B16, tag="oc16")
            nc.vector.memset(ones_c16[:], 1.0)
            any_bias = (flags["has_bqk"] or flags["has_bv"] or flags["has_bo"]
                        or flags["has_b1"] or flags["has_b2"])
            if any_bias:
                ones_r512_32 = cst.tile([1, 512], F32, tag="or512f")
                nc.vector.memset(ones_r512_32[:], 1.0)
                ones_r512_16 = cst.tile([1, 512], B16, tag="or512h")
                nc.vector.memset(ones_r512_16[:], 1.0)
            eps_sb = cst.tile([1, 1], F32, tag="eps")
            nc.vector.memset(eps_sb[:], 1e-5)
            # warm up the Act engine's function table while input DMAs run,
            # so the first real activation doesn't eat the ~2us table load
            warm = cst.tile([1, 1], F32, tag="warm")
            nc.scalar.activation(warm[:], eps_sb[:], AF.Exp)
            if flags["has_bqk"]:
                bqkv_sb = cst.tile([1, 2 * D], F32, tag="bqkv")
                nc.sync.dma_start(bqkv_sb[:], bqkv32[:])
            if (flags["has_bv"] or flags["has_bo"] or flags["has_b1"]
                    or flags["has_b2"]):
                bsml_sb = cst.tile([1, 6 * FF], B16, tag="bsml")
                nc.sync.dma_start(bsml_sb[:], bsml16[:])
            if flags["has_bout"]:
                bout_sb = cst.tile([1, NSP], B16, tag="bout")
                nc.sync.dma_start(bout_sb[:], bout16[:])
            if flags["has_gb"]:
                gb_sb = cst.tile([4, D], F32, tag="gb")
                nc.sync.dma_start(gb_sb[:], gb[:])
                gbp = cst.tile([128, 4, KD], F32, tag="gbp")
                nc.sync.dma_start(
                    gbp[:], gb.rearrange("l (k p) -> p l k", p=128))

            # ---- phase 1: pooling (token-major) + V proj of layer 0 ----
            h32 = acts32.tile([128, KD, NTOK], RT, tag="a32")
            h16 = acts16.tile([128, KD, NTOK], B16, tag="a16", bufs=2)
            v16_l0 = vp.tile([128, BPC, D], B16, tag="v")

            def v_proj_b(b, v16, h16t):
                # V = h @ Wv for one sequence (token-major out)
                for n in range(2):
                    ps = psB.tile([128, 512], F32, tag="big")
                    nsl = slice(n * 512, (n + 1) * 512)
                    for k in range(KD):
                        nc.tensor.matmul(
                            ps[:], h16t[:, k, b * T:(b + 1) * T],
                            wv_sb[:, k, nsl], start=(k == 0),
                            stop=(k == KD - 1 and not flags["has_bv"]))
                    if flags["has_bv"]:
                        nc.tensor.matmul(
                            ps[:], ones_r16[:, :T],
                            bsml_sb[:, 2 * FF + n * 512:2 * FF + (n + 1) * 512],
                            start=False, stop=True)
                    if n == 0:
                        nc.vector.tensor_copy(v16[:, b, nsl], ps[:])
                    else:
                        nc.scalar.copy(v16[:, b, nsl], ps[:])

            for b in range(BPC):
                xt = xa_all[b]
                bsl = slice(b * T, (b + 1) * T)
                psA_ = psS.tile([128, 512], F32, tag="ss")
                for k in range(2):
                    x1, a1 = xt[k]
                    nc.tensor.matmul(psA_[:], a1[:], x1[:, 0:512],
                                     start=(k == 0), stop=(k == 1))
                psB_ = psS.tile([128, 512], F32, tag="ss")
                for k in range(2):
                    x1, a1 = xt[k]
                    nc.tensor.matmul(psB_[:, 0:256], a1[:], x1[:, 512:D_BERT],
                                     start=(k == 0), stop=(k == 1))
                nc.tensor.matmul(psB_[:, 256:512], poh_sb[:, b, :], ptab_sb[:],
                                 start=True, stop=True)
                sA = plsb.tile([128, 512], RT, tag="pl", bufs=3)
                nc.vector.tensor_copy(sA[:], psA_[:])
                sB = plsb.tile([128, 512], RT, tag="pl", bufs=3)
                nc.scalar.copy(sB[:], psB_[:])
                trA = psB.tile([128, 4, 128], RT, tag="big")
                for mm in range(4):
                    nc.tensor.transpose(trA[:, mm, :],
                                        sA[:, mm * 128:(mm + 1) * 128], id32_sb[:])
                trB = psB.tile([128, 4, 128], RT, tag="big")
                for mm in range(4):
                    nc.tensor.transpose(trB[:, mm, :],
                                        sB[:, mm * 128:(mm + 1) * 128], id32_sb[:])
                nc.vector.tensor_copy(h32[:, 0:4, bsl], trA[:])
                nc.scalar.copy(h16[:, 0:4, bsl], trA[:])
                nc.vector.tensor_copy(h32[:, 4:8, bsl], trB[:])
                nc.scalar.copy(h16[:, 4:8, bsl], trB[:])
                v_proj_b(b, v16_l0, h16)
                if b + 1 < BPC:
                    xa_all[b + 1] = emit_input_dmas(b + 1)

            # ---- transformer layers ----
            for li in range(NL):
                first = (li == 0)
                dtq = RT if first else B16
                h_rhs = h32 if first else h16
                wq_d, wk_d = (wq32c, wk32c) if first else (wq16c, wk16c)

                if first:
                    v16 = v16_l0
                else:
                    v16 = vp.tile([128, BPC, D], B16, tag="v")
                    for b in range(BPC):
                        v_proj_b(b, v16, h16)

                ot16 = otp.tile([128, KD, NTOK], B16, tag="ot")
                qkt = {}

                def qkproj(m, half=None):
                    # half: None = emit Q and K, 0 = Q only, 1 = K only
                    parts = ((0, wq_d, "q", True), (1, wk_d, "k", False))
                    if half is not None:
                        parts = (parts[half],)
                    for brow, wd, qk, on_dve in parts:
                        wc = wqkp2.tile([128, KD, 128], dtq, tag="wqk")
                        nc.gpsimd.dma_start(wc[:], wd[m])
                        dst = qkp.tile([128, NTOK], dtq, tag=qk)
                        ps = psB.tile([128, 512], F32, tag="big")
                        for k in range(KD):
                            nc.tensor.matmul(
                                ps[:], wc[:, k, :], h_rhs[:, k, :],
                                start=(k == 0),
                                stop=(k == KD - 1 and not flags["has_bqk"]))
                        if flags["has_bqk"]:
                            nc.tensor.matmul(
                                ps[:], bqkv_sb[:, brow * D + m * 128:
                                               brow * D + (m + 1) * 128],
                                ones_r512_32[:], start=False, stop=True)
                        if on_dve:
                            nc.vector.tensor_copy(dst[:], ps[:])
                        else:
                            nc.scalar.copy(dst[:], ps[:])
                        qkt.setdefault(m, {})[qk] = dst

                qkproj(0)
                for m in range(KD):
                    d_ = qkt.pop(m)
                    qt, kt = d_["q"], d_["k"]
                    ops = psB.tile([128, 512], F32, tag="big")
                    if first and use_pair:
                        # 4 score tiles [128, 2, 256]; interleave the next
                        # qkproj's two halves so the psS ring slot for the
                        # 4th tile is free by the time PE reaches it
                        sst = {}

                        def ss_tile(h2, p):
                            t = psS.tile([128, 2, 256], F32, tag="ss")
                            hsl = slice(64 * h2, 64 * h2 + 64)
                            ksl = slice(p * 256, p * 256 + 256)
                            for j in (0, 1):
                                b = 2 * p + j
                                bsl = slice(b * T, (b + 1) * T)
                                nc.tensor.matmul(t[:, j, :], qt[hsl, bsl],
                                                 kt[hsl, ksl],
                                                 start=True, stop=False)
                                off = 128 * j
                                nc.tensor.matmul(
                                    t[:, j, off:off + 128], ones_r16[:],
                                    bh_sb[:, bsl], start=False, stop=True)
                            sst[(h2, p)] = t

                        ss_tile(0, 0)
                        ss_tile(0, 1)
                        ss_tile(1, 0)
                        if m + 1 < KD:
                            qkproj(m + 1, half=0)
                        ss_tile(1, 1)
                        if m + 1 < KD:
                            qkproj(m + 1, half=1)
                        groups = [(sst[(0, 0)], sst[(0, 1)]),
                                  (sst[(1, 0)], sst[(1, 1)])]
                    else:
                        groups = []
                        for h2 in (0, 1):
                            hsl = slice(64 * h2, 64 * h2 + 64)
                            t = psS.tile([128, BPC, T], F32, tag="ss")
                            for b in range(BPC):
                                bsl = slice(b * T, (b + 1) * T)
                                nc.tensor.matmul(t[:, b, :], qt[hsl, bsl],
                                                 kt[hsl, bsl],
                                                 start=True, stop=False)
                                nc.tensor.matmul(
                                    t[:, b, :], ones_r16[:],
                                    bh_sb[:, bsl], start=False, stop=True)
                            groups.append(t)
                        if m + 1 < KD:
                            qkproj(m + 1)
                    ptps = []
                    for h2 in (0, 1):
                        ex = sfp.tile([128, BPC, T], B16, tag="ex")
                        den4 = sfp.tile([128, BPC], F32, tag="den")
                        skip_max = (not first) and use_nomax
                        if skip_max:
                            # one exp per seq pair (no max bias needed);
                            # per-b denominators via a single DVE reduce
                            t = groups[h2]
                            for p in (0, 1):
                                nc.scalar.activation(
                                    ex[:, 2 * p:2 * p + 2, :],
                                    t[:, 2 * p:2 * p + 2, :], AF.Exp)
                            nc.vector.tensor_reduce(den4[:], ex[:], axis=AX.X,
                                                    op=ALU.add)
                        else:
                            nm4 = sfp.tile([128, BPC], F32, tag="nm")

                            def bsrc(b):
                                if first and use_pair:
                                    p, j = b // 2, b % 2
                                    return groups[h2][p][:, j,
                                                         128 * j:128 * j + 128]
                                return groups[h2][:, b, :]

                            for b in range(BPC):
                                nc.vector.tensor_reduce(
                                    nm4[:, b:b + 1], bsrc(b), axis=AX.X,
                                    op=ALU.max, negate=True)
                            for b in range(BPC):
                                nc.scalar.activation(
                                    ex[:, b, :], bsrc(b), AF.Exp,
                                    bias=nm4[:, b:b + 1], scale=1.0,
                                    accum_out=den4[:, b:b + 1])
                        rcp4 = sfp.tile([128, BPC], F32, tag="rcp")
                        nc.vector.reciprocal(rcp4[:], den4[:])
                        pn = sfp.tile([128, BPC, T], B16, tag="pn")
                        nc.vector.tensor_tensor(
                            pn[:], ex[:],
                            rcp4[:, :, None].to_broadcast((128, BPC, T)),
                            ALU.mult)
                        ptp = psT.tile([128, BPC, T], B16, tag="pt")
                        for b in range(BPC):
                            nc.tensor.transpose(ptp[:, b, :], pn[:, b, :],
                                                id_sb[:])
                        ptps.append(ptp)
                    pts_l = []
                    for h2 in (0, 1):
                        pts = sfp.tile([128, BPC, T], B16, tag="pts")
                        nc.vector.tensor_copy(pts[:], ptps[h2][:])
                        pts_l.append(pts)
                    for h2 in (0, 1):
                        hsl = slice(64 * h2, 64 * h2 + 64)
                        head = 2 * m + h2
                        for b in range(BPC):
                            nc.tensor.matmul(
                                ops[hsl, b * T:(b + 1) * T],
                                v16[:, b, head * 64:(head + 1) * 64],
                                pts_l[h2][:, b, :], start=True, stop=True)
                    if first:
                        nc.scalar.copy(ot16[:, m, :], ops[:])
                    else:
                        nc.vector.tensor_copy(ot16[:, m, :], ops[:])

                # O projection + residual (bf16) + LN1 sum accumulation
                r16 = acts16.tile([128, KD, NTOK], B16, tag="r", bufs=1)
                ps1a = psT.tile([1, 512], F32, tag="pt")
                if not ln1_trick:
                    ps2a = psT.tile([1, 512], F32, tag="pt")
                def ln_hook(ps1, ps2, rsrc, m, last):
                    # PE-side sum accumulation for LN stats; called one
                    # iteration late so the producing DVE chain never
                    # stalls the PE
                    nc.tensor.matmul(ps1[:], ones_c16[:], rsrc[:, m, :],
                                     start=(m == 0), stop=last)
                    if ps2 is not None:
                        sq = lnp.tile([128, 512], B16, tag="sq")
                        nc.vector.tensor_mul(sq[:], rsrc[:, m, :],
                                             rsrc[:, m, :])
                        nc.tensor.matmul(ps2[:], ones_c16[:], sq[:],
                                         start=(m == 0), stop=last)

                for m in range(KD):
                    woc = wstr.tile([128, KD, 128], B16, tag="w")
                    nc.sync.dma_start(woc[:], wo16c[m])
                    ps = psB.tile([128, 512], F32, tag="big")
                    for k in range(KD):
                        nc.tensor.matmul(
                            ps[:], woc[:, k, :], ot16[:, k, :],
                            start=(k == 0),
                            stop=(k == KD - 1 and not flags["has_bo"]))
                    if flags["has_bo"]:
                        nc.tensor.matmul(
                            ps[:], bsml_sb[:, 3 * FF + m * 128:3 * FF + (m + 1) * 128],
                            ones_r512_16[:], start=False, stop=True)
                    if m > 0:
                        ln_hook(ps1a, None if ln1_trick else ps2a, r16,
                                m - 1, False)
                    nc.vector.tensor_tensor(r16[:, m, :], ps[:],
                                            h16[:, m, :], ALU.add)
                ln_hook(ps1a, None if ln1_trick else ps2a, r16, KD - 1, True)

                def bcast16(dst, src, on_dve=False):
                    # [1,512] bf16 -> [128,512] bf16 broadcast
                    if pool_bcast:
                        nc.gpsimd.partition_broadcast(dst[:], src[:])
                    else:
                        bps = psB.tile([128, 512], F32, tag="big")
                        nc.tensor.matmul(bps[:], ones_r16[:], src[:],
                                         start=True, stop=True)
                        if on_dve:
                            nc.vector.tensor_copy(dst[:], bps[:])
                        else:
                            nc.scalar.copy(dst[:], bps[:])

                def ln_tail(ps1, ps2, vin16, g_idx, b_idx, out_tag):
                    """Full LN: vin16 (bf16 residual) -> normalized bf16."""
                    mu = lns.tile([1, 512], F32, tag="mu")
                    nc.vector.tensor_scalar_mul(mu[:], ps1[:], 1.0 / D)
                    m2 = lns.tile([1, 512], F32, tag="m2")
                    nc.vector.tensor_mul(m2[:], mu[:], mu[:])
                    tmp = lns.tile([1, 512], F32, tag="tmp")
                    nc.vector.scalar_tensor_tensor(
                        tmp[:], ps2[:], 1.0 / D, m2[:],
                        ALU.mult, ALU.subtract)
                    nc.scalar.activation(tmp[:], tmp[:], AF.Ln,
                                         bias=eps_sb[:], scale=1.0)
                    rstd = lns.tile([1, 512], B16, tag="rstd")
                    nc.scalar.activation(rstd[:], tmp[:], AF.Exp, scale=-0.5)
                    mrs = lns.tile([1, 512], B16, tag="mrs")
                    nc.vector.scalar_tensor_tensor(
                        mrs[:], mu[:], -1.0, rstd[:], ALU.mult, ALU.mult)
                    rsb = lnb.tile([128, 512], B16, tag="rsb")
                    bcast16(rsb, rstd)
                    mrsb = lnb.tile([128, 512], B16, tag="mrsb")
                    bcast16(mrsb, mrs, on_dve=True)
                    o16 = acts16.tile([128, KD, NTOK], B16, tag=out_tag,
                                      bufs=2 if out_tag == "a16" else 1)
                    for k in range(KD):
                        t = lnp.tile([128, 512], B16, tag="t")
                        nc.vector.tensor_mul(t[:], vin16[:, k, :], rsb[:])
                        nc.vector.tensor_tensor(o16[:, k, :], t[:], mrsb[:],
                                                ALU.add)
                        if flags["has_gb"]:
                            nc.vector.tensor_scalar(
                                o16[:, k, :], o16[:, k, :],
                                gbp[:, g_idx, k:k + 1], gbp[:, b_idx, k:k + 1],
                                ALU.mult, ALU.add)
                    return o16

                if ln1_trick:
                    # LN1 reduces to mean-centering (scale/shift cancel in LN2)
                    mu16 = lns.tile([1, 512], B16, tag="mu16")
                    nc.vector.tensor_scalar_mul(mu16[:], ps1a[:], 1.0 / D)
                    mub = lnb.tile([128, 512], B16, tag="mub")
                    bcast16(mub, mu16)
                    h1_16 = acts16.tile([128, KD, NTOK], B16, tag="h1")
                    for k in range(KD):
                        nc.vector.tensor_sub(h1_16[:, k, :], r16[:, k, :],
                                             mub[:])
                else:
                    h1_16 = ln_tail(ps1a, ps2a, r16, 0, 1, "h1")

                # FFN
                ft = ftp.tile([128, MF, NTOK], B16, tag="ft")
                for mf in range(MF):
                    w1c = wstr.tile([128, KD, 128], B16, tag="w")
                    nc.sync.dma_start(w1c[:], w116c[mf])
                    ps = psB.tile([128, 512], F32, tag="big")
                    for k in range(KD):
                        nc.tensor.matmul(
                            ps[:], w1c[:, k, :], h1_16[:, k, :],
                            start=(k == 0),
                            stop=(k == KD - 1 and not flags["has_b1"]))
                    if flags["has_b1"]:
                        nc.tensor.matmul(
                            ps[:], bsml_sb[:, 5 * FF + mf * 128:5 * FF + (mf + 1) * 128],
                            ones_r512_16[:], start=False, stop=True)
                    nc.scalar.activation(ft[:, mf, :], ps[:], AF.Relu)
                r2 = acts16.tile([128, KD, NTOK], B16, tag="r", bufs=1)
                ps1b = psT.tile([1, 512], F32, tag="pt")
                ps2b = psT.tile([1, 512], F32, tag="pt")
                for m in range(KD):
                    w2c = wstr.tile([128, MF, 128], B16, tag="w")
                    nc.sync.dma_start(w2c[:], w216c[m])
                    ps = psB.tile([128, 512], F32, tag="big")
                    for k in range(MF):
                        nc.tensor.matmul(
                            ps[:], w2c[:, k, :], ft[:, k, :],
                            start=(k == 0),
                            stop=(k == MF - 1 and not flags["has_b2"]))
                    if flags["has_b2"]:
                        nc.tensor.matmul(
                            ps[:], bsml_sb[:, 4 * FF + m * 128:4 * FF + (m + 1) * 128],
                            ones_r512_16[:], start=False, stop=True)
                    if m > 0:
                        ln_hook(ps1b, ps2b, r2, m - 1, False)
                    nc.vector.tensor_tensor(r2[:, m, :], ps[:],
                                            h1_16[:, m, :], ALU.add)
                ln_hook(ps1b, ps2b, r2, KD - 1, True)

                if li == NL - 1:
                    # prefetch first vocab weight chunk behind the LN tail
                    woc_v0 = wstr.tile([128, KD, 512], B16, tag="w")
                    nc.gpsimd.dma_start(woc_v0[:], woutc[0])
                h16 = ln_tail(ps1b, ps2b, r2, 2, 3, "a16")

            # ---- final vocab projection (token-major) ----
            for n in range(NCH):
                ncols = 512 if n < NCH - 1 else NSENSE - 512 * (NCH - 1)
                if n == 0:
                    woc = woc_v0
                else:
                    woc = wstr.tile([128, KD, 512], B16, tag="w")
                    nc.gpsimd.dma_start(woc[:, :, :ncols], woutc[n][:, :, :ncols])
                for mt in range(BPC):
                    tsl = slice(mt * 128, (mt + 1) * 128)
                    ps = psB.tile([128, 512], F32, tag="big")
                    for k in range(KD):
                        nc.tensor.matmul(
                            ps[:, :ncols], h16[:, k, tsl], woc[:, k, :ncols],
                            start=(k == 0),
                            stop=(k == KD - 1 and not flags["has_bout"]))
                    if flags["has_bout"]:
                        nc.tensor.matmul(
                            ps[:, :ncols], ones_r16[:],
                            bout_sb[:, n * 512:n * 512 + ncols],
                            start=False, stop=True)
                    lg = evp.tile([128, 512], F32, tag="lg")
                    if mt % 2 == 0:
                        nc.scalar.copy(lg[:, :ncols], ps[:, :ncols])
                    else:
                        nc.vector.tensor_copy(lg[:, :ncols], ps[:, :ncols])
                    nc.sync.dma_start(out[tsl, n * 512:n * 512 + ncols],
                                      lg[:, :ncols])

    _split_multi_waits(nc)
    nc.finalize()
    return nc


# ---------------------------------------------------------------------------
# Host-side prep + run
# ---------------------------------------------------------------------------

def _prep(inputs):
    """Build per-core in_maps from full inputs."""
    x = np.asarray(inputs["x"], np.float32)
    word_ids = np.asarray(inputs["word_ids"], np.int32)
    text_lengths = np.asarray(inputs["text_lengths"], np.int32)
    pos_tags = np.asarray(inputs["pos_tags"], np.int64)
    pos_table = np.asarray(inputs["pos_table"], np.float32)

    # pooling matrix A[b, s, t] = SCALE / cnt[b, t] if word_ids[b,s]==t
    cnt = np.zeros((B, T), np.float32)
    np.add.at(cnt, (np.arange(B)[:, None], word_ids), 1.0)
    cntc = np.maximum(cnt, 1.0)
    A = np.zeros((B, S, T), np.float32)
    bi = np.repeat(np.arange(B), S)
    si = np.tile(np.arange(S), B)
    ti = word_ids.ravel()
    A[bi, si, ti] = SCALE / cntc[bi, ti]

    # pos one-hot (padded to 32 rows) x SCALE
    poh = np.zeros((B, 32, T), np.float32)
    poh[np.repeat(np.arange(B), T), pos_tags.ravel().astype(np.int64),
        np.tile(np.arange(T), B)] = SCALE
    ptab = np.zeros((32, D_POS), np.float32)
    ptab[:NPOS] = pos_table

    key_mask = np.arange(T)[None, :] < text_lengths[:, None]
    bias_row = np.where(key_mask, 0.0, -1e9).astype(np.float32)

    Wqs = (np.asarray(inputs["Wq"], np.float32) * ATTN_SCALE)
    Wk = np.asarray(inputs["Wk"], np.float32)
    Wv = np.asarray(inputs["Wv"], np.float32)
    Wo = np.asarray(inputs["Wo"], np.float32)
    W1 = np.asarray(inputs["W1"], np.float32)
    W2 = np.asarray(inputs["W2"], np.float32)
    Wout = np.asarray(inputs["Wout"], np.float32)

    def colchunk(w, asdt):
        din, dout = w.shape
        return np.ascontiguousarray(
            w.reshape(din // 128, 128, dout // 128, 128).transpose(2, 1, 0, 3)
        ).astype(asdt)

    wq32c = colchunk(Wqs, np.float32)
    wk32c = colchunk(Wk, np.float32)
    wq16c = colchunk(Wqs, BF16)
    wk16c = colchunk(Wk, BF16)
    wo16c = colchunk(Wo, BF16)
    w116c = colchunk(W1, BF16)
    w216c = colchunk(W2, BF16)
    wv16r = np.ascontiguousarray(
        Wv.reshape(KD, 128, D).transpose(1, 0, 2)).astype(BF16)
    Wout_p = np.zeros((D, NSP), np.float32)
    Wout_p[:, :NSENSE] = Wout
    woutc = np.ascontiguousarray(
        Wout_p.reshape(KD, 128, NCH, 512).transpose(2, 1, 0, 3)).astype(BF16)

    bq = np.asarray(inputs["bq"], np.float32) * ATTN_SCALE
    bk = np.asarray(inputs["bk"], np.float32)
    bqkv32 = np.stack([bq, bk]).astype(np.float32).reshape(1, 2 * D)
    bsml = np.zeros((6, FF), np.float32)
    bsml[2, :D] = np.asarray(inputs["bv"], np.float32)
    bsml[3, :D] = np.asarray(inputs["bo"], np.float32)
    bsml[4, :D] = np.asarray(inputs["b2"], np.float32)
    bsml[5] = np.asarray(inputs["b1"], np.float32)
    bout = np.zeros((1, NSP), np.float32)
    bout[0, :NSENSE] = np.asarray(inputs["bout"], np.float32)
    gbarr = np.stack([np.asarray(inputs["ln1_g"], np.float32),
                      np.asarray(inputs["ln1_b"], np.float32),
                      np.asarray(inputs["ln2_g"], np.float32),
                      np.asarray(inputs["ln2_b"], np.float32)])

    flags = {
        "has_bqk": bool(np.any(bqkv32)),
        "has_bv": bool(np.any(bsml[2])),
        "has_bo": bool(np.any(bsml[3])),
        "has_b2": bool(np.any(bsml[4])),
        "has_b1": bool(np.any(bsml[5])),
        "has_bout": bool(np.any(bout)),
        "has_gb": bool(np.any(gbarr[1]) or np.any(gbarr[3])
                       or not np.all(gbarr[0] == 1.0)
                       or not np.all(gbarr[2] == 1.0)),
    }

    ident16 = np.eye(128, dtype=np.float32).astype(BF16)
    ident32 = np.eye(128, dtype=np.float32)

    shared = dict(
        wq32c=wq32c, wk32c=wk32c, wq16c=wq16c, wk16c=wk16c, wo16c=wo16c,
        w116c=w116c, w216c=w216c, wv16r=wv16r, woutc=woutc, ptab=ptab,
        id16=ident16, id32=ident32, bqkv32=bqkv32,
        bsml16=bsml.astype(BF16).reshape(1, 6 * FF),
        bout16=bout.astype(BF16), gb=gbarr,
    )

    in_maps = []
    for c in range(NCORES):
        bsl = slice(c * BPC, (c + 1) * BPC)
        m = dict(shared)
        m["xw"] = np.ascontiguousarray(
            x[bsl].reshape(BPC, 2, 128, D_BERT))
        m["aw"] = np.ascontiguousarray(A[bsl].reshape(BPC, 2, 128, T))
        m["poh"] = np.ascontiguousarray(poh[bsl].transpose(1, 0, 2))
        m["biash"] = np.ascontiguousarray(bias_row[bsl]).reshape(1, BPC * T).astype(BF16)
        in_maps.append(m)
    return in_maps, flags


def kernel(**inputs) -> np.ndarray:
    in_maps, flags = _prep(inputs)
    key = (os.environ.get("KPAIR", "1"), os.environ.get("KNOMAX", "1"),
           os.environ.get("KTRICK", "1"), os.environ.get("KPOOLB", "0"),
           ) + tuple(sorted(flags.items()))
    if key not in _BUILD_CACHE:
        _BUILD_CACHE[key] = _build(flags)
    nc = _BUILD_CACHE[key]

    if os.environ.get("KERNEL_SIM") == "1":
        from concourse.bass_interp import CoreSim
        ncore = int(os.environ.get("KERNEL_SIM_CORES", "1"))
        outs = []
        for c in range(ncore):
            sim = CoreSim(nc)
            for name, arr in in_maps[c].items():
                sim.tensor(name)[:] = arr
            sim.simulate()
            outs.append(np.asarray(sim.tensor("out")).copy())
        full = np.zeros((B, T, NSENSE), np.float32)
        for c in range(ncore):
            full[c * BPC:(c + 1) * BPC] = outs[c].reshape(BPC, T, NSENSE)
        return full

    from concourse.bass_utils import run_bass_kernel_spmd
    r = run_bass_kernel_spmd(nc, in_maps, core_ids=list(range(NCORES)))
    full = np.concatenate(
        [r.results[c]["out"].reshape(BPC, T, NSENSE) for c in range(NCORES)],
        axis=0)
    return full
